# revision 48
# baseline (speedup 1.0000x reference)
"""GAT message-passing kernel for TRN2 (8-core SPMD), v2.

Math (heads h, nodes n):
  t[n,h,:] = x[n] @ Ws[h].T            (t-space features, 64 per head)
  Ar[n,h]  = x[n] @ war[:,h]           (war = Ws[h].T @ a_r[h], folded weights)
  u        = exp(Ar)
  out[i, h*64:h*64+64] = elu( sum_{e:src=i} u[dst,h]*t[dst,h,:] / sum u[dst,h] )

Sharding: src-range per core (6272 nodes = 49 windows of 128).

Phase 1 (build Y): Y row = [t~ (512 bf16) | u (8 bf16) | pad (8)] = 1056 B.
Y is stored partition-major per half: node n -> (p = n%128, tile = n//128),
row index r = p*196 + (tile - 196h), so phase-1 writes go out as chunked
[128, 14, 528] DMAs (14.8 KB/descriptor) instead of per-row scribbles.
x^T is loaded in matching 14-tile chunks on the scalar (ACT) HWDGE queue.

Phase 2 (gather + segment-sum): per-edge Y rows gathered via gpsimd
dma_gather (rows sorted by dst within each (window, half) group for DRAM
locality; trailing pad slots idx=-1 are skipped by HW). One-hot S (fp8,
built by DVE is_equal against iota) scatter-sums 128-edge blocks into PSUM
via PE matmul: one 512-col matmul (t~) + one 8-col matmul (u) per block.
Two passes (dst half 0, then half 1) accumulate into an SBUF accumulator,
so pass-0 gathers only fence on the half-0 table writes and overlap the
second half of phase 1. Pass 1 evicts: alpha-normalize + elu + store.
"""

import math
import numpy as np
from contextlib import ExitStack

import concourse.bass as bass
import concourse.bacc as bacc
import concourse.mybir as mybir
from concourse.tile import TileContext
from concourse.tile import add_dep_helper

F32 = mybir.dt.float32
BF16 = mybir.dt.bfloat16
FP8 = mybir.dt.float8e4
I16 = mybir.dt.int16

P = 128
IN_FEAT = 256
HEADS = 8
OUT = 64
TD = HEADS * OUT       # 512
YW_ROW = 640           # Y row stride in elements (1280 B, mult of 256)
YW = 528               # gathered elements per row: 512 t~ + 8 u + 8 slack
                       # (1056 B per gathered row, 32B-aligned SBUF stride)
W_PER_CORE = 49        # windows (128 src rows) per core
NPC = W_PER_CORE * P   # 6272 nodes per core
N_CORES = 8
N_PAD = NPC * N_CORES  # 50176 padded nodes
N_TILES = N_PAD // P   # 392
T_HALF = N_TILES // 2  # 196 tiles per dst half
H_ROWS = T_HALF * P    # 25088 rows per half table
TC = 14                # phase-1 tiles per chunk write
NCHUNK = N_TILES // TC // 2  # 14 chunks per half
NB_MAX = 10            # gather blocks per call (multi-packet; fewer calls
                       # cuts serial GpSimd per-call overhead)

import os as _os
# "raw" = custom dma_gather ucode, elem 528 of 640-stride rows (the Q7
# desc-gen ucode runs ~5.6 ns/row which, with HBM random-read latency,
# caps the gather around 210-225 GB/s -- measured best overall).
# "indirect" = stock dynamic-DMA (InstDMACopy) row gather with dense
# 528-col rows: the HW offsets layout expects an engine-spray swizzle
# (NOT plain [p, blk]); produced garbage + 37 ms on HW -- do not use
# without reworking the host-side offsets packing to the spray order.
GATHER_MODE = _os.environ.get("GATHER_MODE", "raw")
NB_IND = 16            # blocks per indirect gather call


class Config:
    def __init__(self, n_nodes, src, dst):
        assert n_nodes <= N_PAD
        src = np.asarray(src, dtype=np.int64)
        dst = np.asarray(dst, dtype=np.int64)
        core = src // NPC
        w = (src % NPC) // P
        lsrc = src % P
        half = (dst >= H_ROWS).astype(np.int64)
        # partition-major Y row index within the half table
        lidx = (dst % P) * T_HALF + (dst // P) - T_HALF * half

        counts = np.zeros((N_CORES, W_PER_CORE, 2), dtype=np.int64)
        np.add.at(counts, (core, w, half), 1)
        cap = counts.max(axis=0)           # [W, 2] max over cores
        cap = np.maximum(cap, 1)           # ensure >=1 block per (w, half)
        self.cap_blocks = np.ceil(cap / P).astype(np.int64)
        self.tot_blocks = int(self.cap_blocks.sum())
        self.tot_idx = self.tot_blocks * P

        # order edges: key = (core, half, w, dst); dst-sorted within group
        # for DRAM gather locality. np.lexsort: last key is primary.
        order = np.lexsort((dst, w, half, core))
        s_core, s_w, s_half = core[order], w[order], half[order]
        s_lsrc, s_lidx = lsrc[order], lidx[order]

        # block offsets per (w, half) in the packed per-core slot stream;
        # stream is ordered (half, w) to match pass structure
        blk_off = np.zeros((W_PER_CORE, 2), dtype=np.int64)
        acc = 0
        for hi in range(2):
            for wi in range(W_PER_CORE):
                blk_off[wi, hi] = acc
                acc += self.cap_blocks[wi, hi]
        self.blk_off = blk_off

        # rank within each (core, half, w) group (groups contiguous after sort)
        gkey = (s_core * 2 + s_half) * W_PER_CORE + s_w
        change = np.r_[True, gkey[1:] != gkey[:-1]]
        grp_start = np.flatnonzero(change)
        grp_id = np.cumsum(change) - 1
        grp_rank = np.arange(len(order)) - grp_start[grp_id]
        slot = blk_off[s_w, s_half] * P + grp_rank

        # idx column split point between the two halves (blocks are packed
        # half-major, so each half's idx columns are contiguous)
        self.h_col = [0, int(blk_off[0, 1]) * 8, self.tot_blocks * 8]
        self.h_blk = [0, int(blk_off[0, 1]), self.tot_blocks]

        # call table per (w, half): chunks of <= NB_MAX blocks
        self.calls = {}   # (w, h) -> list of (b0, nb)
        for hi in range(2):
            for wi in range(W_PER_CORE):
                c = int(self.cap_blocks[wi, hi])
                b0 = int(blk_off[wi, hi])
                lst = []
                off = 0
                while off < c:
                    nb = min(NB_MAX, c - off)
                    lst.append((b0 + off, nb))
                    off += nb
                self.calls[(wi, hi)] = lst

        # idx packed [16, tot_idx/16] call-granular wrap, replicated to 128;
        # pad slots get -1 (skipped by HW: negative idxs at call end ignored)
        self.idx_packed = np.full((N_CORES, 128, self.tot_idx // 16), -1,
                                  np.int16)
        self.meta_packed = np.full((N_CORES, P, self.tot_blocks), -1.0,
                                   np.float32)
        call_starts = []
        for hi in range(2):
            for wi in range(W_PER_CORE):
                for (b0, nb) in self.calls[(wi, hi)]:
                    call_starts.append(b0 * P)
        call_starts = np.array(sorted(call_starts), dtype=np.int64)
        ci = np.searchsorted(call_starts, slot, side="right") - 1
        g0 = call_starts[ci]
        i_in_call = slot - g0
        row16 = i_in_call % 16
        col16 = g0 // 16 + i_in_call // 16
        self.idx_packed[s_core, row16, col16] = s_lidx.astype(np.int16)
        self.idx_packed[:, 16:, :] = np.tile(
            self.idx_packed[:, :16, :], (1, 7, 1))
        blk = slot // P
        pslot = slot % P
        self.meta_packed[s_core, pslot, blk] = s_lsrc.astype(np.float32)

        # int32 row-index offsets for the indirect-DMA gather:
        # slot (blk*128 + p) -> offsets[p, blk]; pad slots read row 0
        self.idx32_packed = np.zeros((N_CORES, 128, self.tot_blocks),
                                     np.int32)
        self.idx32_packed[s_core, pslot, blk] = s_lidx.astype(np.int32)

        # groups with zero real edges keep idx -1 everywhere: the gather
        # ucode trims trailing negatives (possibly to zero descriptors) but
        # still pushes the completion-semaphore descriptor, so empty calls
        # are safe; meta=-1 keeps the one-hot all-zero.
        if GATHER_MODE in ("stock0", "raw"):
            # negative-idx trimming crashes the device (NRT_EXEC_UNIT_
            # UNRECOVERABLE, verified on HW) -> pad slots gather row 0
            # harmlessly; meta=-1 keeps them out of the one-hot.
            self.idx_packed = np.maximum(self.idx_packed, 0)


def dma_gather_raw(gp, out_ap, in_ap, idxs_ap, num_idxs, num_idxs_reg,
                   elem_size, elem_step, single_packet, queue_num):
    """nc.gpsimd.dma_gather for DRAM sources, minus the elem_size%256
    assert. The gather ucode handles arbitrary elem_size (partial last
    packet); only the row stride (elem_step bytes) must be a multiple of
    256 since it's encoded as stride_bytes_256."""
    from concourse.bass import MemorySpace, ap_utils, exact_div

    gp._assert_queue_num(queue_num)
    assert idxs_ap.dtype == mybir.dt.int16
    assert in_ap.space == MemorySpace.DRAM
    assert in_ap.dtype == out_ap.dtype
    assert idxs_ap.space == MemorySpace.SBUF
    assert out_ap.space == MemorySpace.SBUF
    assert ap_utils.ap_is_contiguous(in_ap.ap[1:])
    assert ap_utils.ap_is_contiguous(out_ap.ap[1:])
    assert ap_utils.ap_is_contiguous(idxs_ap.ap[1:])
    assert in_ap.ap[-1][1] == out_ap.ap[-1][1] == elem_size
    from concourse.bass import round_up_to_multiple
    assert out_ap.ap[0][1] * out_ap.ap[1][1] == round_up_to_multiple(
        num_idxs, 128)
    assert in_ap.ap[0][0] == elem_step
    stride_bytes = elem_step * mybir.dt.size(in_ap.dtype)
    stride_bytes_256 = exact_div(stride_bytes, 256)
    assert stride_bytes_256 < 256

    _in_ap = gp.lower_ap_dma(in_ap, for_custom_bir_dma=True)
    _idxs_ap = gp.lower_ap(idxs_ap)
    _out_ap = gp.lower_ap(out_ap)
    inst = gp.add_instruction(
        mybir.InstDMAGatherAnt(
            name=gp.bass.get_next_instruction_name(),
            ins=[
                *_in_ap,
                _idxs_ap,
                gp.lower_val_access(gp.to_reg(num_idxs_reg)),
            ],
            outs=[_out_ap],
            transpose=False,
            num_idxs=num_idxs,
            elem_size=elem_size,
            stride_bytes_256=stride_bytes_256,
            gen_mode=0,
            single_packet=single_packet,
            queue_num=queue_num,
            sbuf_tokens_per_rank=0,
            sbuf_free_dim_per_rank=0,
            sbuf_free_dim_pad_per_rank=0,
            sbuf_byte_offset=0,
        )
    )
    return inst


def indirect_gather_q(gp, out, in_, offs_ap, queue_num):
    """nc.gpsimd.indirect_dma_start (in_offset case) with a selectable
    SWDGE queue. Gathers in_[offs[p, b], :] -> out[p, b, :] via the stock
    dynamic-DMA descriptor path (vectorized Q7 codegen)."""
    from concourse.bass import MemorySpace

    assert in_.space == MemorySpace.DRAM
    assert out.space == MemorySpace.SBUF
    assert isinstance(in_.offset, int) and in_.offset == 0
    out_l = gp.lower_ap_dma(out, for_indirect_dma=True)
    in_l = gp.lower_ap_dma(in_, for_indirect_dma=True)
    assert len(in_l) == 1 and len(out_l) == 1
    off_l = gp.lower_ap_dma(offs_ap)
    assert len(off_l) == 1
    in_l.append(off_l[0])

    ap_shape = in_.shape
    coef = 1
    for i in range(1, len(ap_shape)):
        coef *= ap_shape[i]
    in_l[0].dynamic_ap_info = mybir.DynamicAccessPatternInfo(
        c=0,
        actual_ap=out.ap,
        indirect_dim_max_index=ap_shape[0],
        offset_expr=[
            mybir.DynamicAccessPatternOffsetExpr(
                coef=coef,
                aff_expr=mybir.DynamicAccessPatternOffsetExprAffExpr(
                    kind="IndirectArgId", arg_id=1),
            )
        ],
    )
    qname = f"qPoolDynamic{queue_num or ''}"
    inst = gp.add_instruction(
        mybir.InstDMACopy(
            name=gp.bass.get_next_instruction_name(),
            queue=qname,
            mode="Copy",
            ins=in_l,
            outs=out_l,
            oob_is_err=True,
            cce_op=mybir.AluOpType.bypass,
        )
    )
    return inst


def build_program(cfg: Config):
    nc = bacc.Bacc("TRN2", target_bir_lowering=False, debug=False,
                   num_devices=N_CORES, num_swdge_queues=4)

    indirect = GATHER_MODE == "indirect"
    y_stride = YW if indirect else YW_ROW  # dense rows for indirect gather

    xt_d = nc.dram_tensor("xt", [128, N_TILES, 2, P], BF16,
                          kind="ExternalInput")
    wcat_d = nc.dram_tensor("wcat", [IN_FEAT, TD], BF16, kind="ExternalInput")
    war_d = nc.dram_tensor("war", [IN_FEAT, HEADS], BF16,
                           kind="ExternalInput")
    iota_d = nc.dram_tensor("iota", [P, P], BF16, kind="ExternalInput")
    if indirect:
        idx_d = nc.dram_tensor("idx32", [128, cfg.tot_blocks], mybir.dt.int32,
                               kind="ExternalInput")
    else:
        idx_d = nc.dram_tensor("idx", [128, cfg.tot_idx // 16], I16,
                               kind="ExternalInput")
    meta_d = nc.dram_tensor("meta", [P, cfg.tot_blocks], BF16,
                            kind="ExternalInput")
    out_d = nc.dram_tensor("out", [128, W_PER_CORE, TD], F32,
                           kind="ExternalOutput")
    y_d = [nc.dram_tensor("y0", [H_ROWS, y_stride], BF16, kind="Internal"),
           nc.dram_tensor("y1", [H_ROWS, y_stride], BF16, kind="Internal")]

    y_writes = [[], []]  # per half
    with TileContext(nc) as tc:
        with ExitStack() as ctx:
            # ---------------- persistent tiles ----------------
            consts = ctx.enter_context(tc.tile_pool(name="consts", bufs=1))
            wc = consts.tile([P, 2, TD], BF16, tag="wc")
            nc.sync.dma_start(wc[:, :, :],
                              wcat_d.rearrange("(k p) n -> p k n", p=P))
            wr = consts.tile([P, 2, HEADS], BF16, tag="wr")
            nc.sync.dma_start(wr[:, :, :],
                              war_d.rearrange("(k p) n -> p k n", p=P))
            iota = consts.tile([P, P], BF16, tag="iota")
            nc.sync.dma_start(iota[:, :], iota_d[:, :])
            meta_sb = consts.tile([P, cfg.tot_blocks], BF16, tag="meta")
            nc.sync.dma_start(meta_sb[:, :], meta_d[:, :])
            neg1 = consts.tile([P, 1], F32, tag="neg1")
            nc.vector.memset(neg1[:, :], -1.0)
            acc_t = consts.tile([P, W_PER_CORE, TD], BF16, tag="acc_t")
            acc_u = consts.tile([P, W_PER_CORE, HEADS], F32, tag="acc_u")

            # ---------------- phase 1: build Y table ----------------
            xin = ctx.enter_context(tc.tile_pool(name="xin", bufs=2))
            stgp = ctx.enter_context(tc.tile_pool(
                name="stg", bufs=3 if GATHER_MODE == "raw" else 2))
            idxp = ctx.enter_context(tc.tile_pool(name="idxp", bufs=1))
            ps_t = ctx.enter_context(tc.tile_pool(name="ps_t", bufs=2,
                                                  space="PSUM"))
            ps_a = ctx.enter_context(tc.tile_pool(name="ps_a", bufs=2,
                                                  space="PSUM"))

            def emit_chunk(c):
                xT = xin.tile([128, TC, 2, P], BF16)
                # loads ride the ring opposite to this chunk's write
                leng = nc.scalar if c % 2 == 0 else nc.sync
                leng.dma_start(xT[:, :, :, :],
                               xt_d[:, c * TC:(c + 1) * TC, :, :])
                stg = stgp.tile([P, TC, y_stride], BF16)
                for tt in range(TC):
                    pt = ps_t.tile([P, TD], F32, tag="pt")
                    par = ps_a.tile([P, HEADS], F32, tag="par")
                    nc.tensor.matmul(par[:, :], xT[:, tt, 0, :], wr[:, 0, :],
                                     start=True, stop=False)
                    nc.tensor.matmul(par[:, :], xT[:, tt, 1, :], wr[:, 1, :],
                                     start=False, stop=True)
                    nc.tensor.matmul(pt[:, :], xT[:, tt, 0, :], wc[:, 0, :],
                                     start=True, stop=False)
                    nc.tensor.matmul(pt[:, :], xT[:, tt, 1, :], wc[:, 1, :],
                                     start=False, stop=True)
                    nc.scalar.activation(
                        stg[:, tt, TD:TD + HEADS], par[:, :],
                        mybir.ActivationFunctionType.Exp)
                    nc.vector.tensor_tensor(
                        stg[:, tt, 0:TD].rearrange("p (h o) -> p h o",
                                                   h=HEADS),
                        pt[:, :].rearrange("p (h o) -> p h o", h=HEADS),
                        stg[:, tt, TD:TD + HEADS].unsqueeze(2).broadcast_to(
                            [P, HEADS, OUT]),
                        mybir.AluOpType.mult,
                    )
                half = 0 if c < NCHUNK else 1
                cc = c % NCHUNK
                dst_ap = y_d[half].rearrange("(p t) e -> p t e", p=P)[
                    :, cc * TC:(cc + 1) * TC, :]
                # alternate chunk writes across the two HWDGE rings
                eng = nc.sync if c % 2 == 0 else nc.scalar
                wi_ = eng.dma_start(dst_ap, stg[:, :, :])
                y_writes[half].append(wi_)

            # all chunks up front: interleaving half-1 chunks into the
            # gather stream was tried and regressed (in-order engines turn
            # any phase-1 stall into gather-pipeline head-of-line blocking,
            # and writes steal gather bandwidth ~1:1 regardless of pacing)
            for c in range(2 * NCHUNK):
                emit_chunk(c)

            # ---------------- phase 2: gather + segment sums ----------------
            gpool = ctx.enter_context(tc.tile_pool(
                name="gath", bufs=3 if GATHER_MODE == "indirect" else 4))
            spool = ctx.enter_context(tc.tile_pool(name="onehot", bufs=4))
            opool = ctx.enter_context(tc.tile_pool(name="outp", bufs=2))
            ps_nt = ctx.enter_context(tc.tile_pool(name="ps_nt", bufs=2,
                                                   space="PSUM"))
            ps_nu = ctx.enter_context(tc.tile_pool(name="ps_nu", bufs=2,
                                                   space="PSUM"))

            fence_pending = [True, True]
            qn_box = [0]
            if indirect:
                idx32_sb = idxp.tile([128, cfg.tot_blocks], mybir.dt.int32,
                                     tag="idx32")
                nc.sync.dma_start(idx32_sb[:, :], idx_d[:, :])
            for h in range(2):
                if indirect:
                    idx_sb = idx32_sb
                else:
                    c0, c1 = cfg.h_col[h], cfg.h_col[h + 1]
                    idx_sb = idxp.tile([128, c1 - c0], I16)
                    nc.sync.dma_start(idx_sb[:, :], idx_d[:, c0:c1])

                # indirect mode: fixed NB_IND-block calls spanning windows
                # within the half; (g, s) tiles are consumed across windows
                cur = {"g": None, "s": None, "base": 0, "nb": 0}

                def emit_call(k0):
                    nb = min(NB_IND, cfg.h_blk[h + 1] - k0)
                    g = gpool.tile([P, NB_IND, YW], BF16)
                    g_inst = indirect_gather_q(
                        nc.gpsimd,
                        out=g[:, 0:nb, :],
                        in_=y_d[h][:, :],
                        offs_ap=idx_sb[:, k0:k0 + nb],
                        queue_num=qn_box[0],
                    )
                    qn_box[0] = (qn_box[0] + 1) % 4
                    if fence_pending[h]:
                        # gather's indexed read of Y is invisible to Tile dep
                        # tracking; gathers run in order on GpSimd, so gating
                        # the first gather per half on that half's chunk
                        # writes fences all of them.
                        for wr_ in y_writes[h]:
                            add_dep_helper(g_inst.ins, wr_.ins,
                                           reason="gather reads Y table")
                        fence_pending[h] = False
                    s = spool.tile([P, NB_IND, P], FP8)
                    nc.vector.tensor_tensor(
                        s[:, 0:nb, :],
                        meta_sb[:, k0:k0 + nb].unsqueeze(2).broadcast_to(
                            [P, nb, P]),
                        iota[:, :].unsqueeze(1).broadcast_to([P, nb, P]),
                        mybir.AluOpType.is_equal,
                    )
                    cur["g"], cur["s"], cur["base"], cur["nb"] = g, s, k0, nb

                for w in range(W_PER_CORE):
                    nblk_w = int(cfg.cap_blocks[w, h])
                    wb0 = int(cfg.blk_off[w, h])
                    pn_t = ps_nt.tile([P, TD], F32, tag="pn_t")
                    pn_u = ps_nu.tile([P, HEADS], F32, tag="pn_u")
                    if indirect:
                        for bi in range(nblk_w):
                            blk = wb0 + bi
                            if (cur["g"] is None
                                    or blk >= cur["base"] + cur["nb"]):
                                emit_call(blk)
                            j = blk - cur["base"]
                            g, s = cur["g"], cur["s"]
                            st = (bi == 0)
                            sp = (bi == nblk_w - 1)
                            nc.tensor.matmul(pn_t[:, :], s[:, j, :],
                                             g[:, j, 0:TD],
                                             start=st, stop=sp,
                                             skip_group_check=True)
                            nc.tensor.matmul(pn_u[:, :], s[:, j, :],
                                             g[:, j, TD:TD + HEADS],
                                             start=st, stop=sp,
                                             skip_group_check=True)
                    else:
                        bi = 0
                        for (b0, nb) in cfg.calls[(w, h)]:
                            g = gpool.tile([P, NB_MAX, YW], BF16)
                            g_inst = dma_gather_raw(
                                nc.gpsimd,
                                out_ap=g[:, 0:nb, :],
                                in_ap=y_d[h][:, 0:YW],
                                idxs_ap=idx_sb[:, b0 * 8 - c0:
                                               (b0 + nb) * 8 - c0],
                                num_idxs=nb * P,
                                num_idxs_reg=nb * P,
                                elem_size=YW,
                                elem_step=YW_ROW,
                                single_packet=(nb * P <= 1024),
                                queue_num=qn_box[0],
                            )
                            qn_box[0] = (qn_box[0] + 1) % 4
                            if fence_pending[h]:
                                for wr_ in y_writes[h]:
                                    add_dep_helper(g_inst.ins, wr_.ins,
                                                   reason="gather reads Y")
                                fence_pending[h] = False
                            s = spool.tile([P, NB_MAX, P], FP8)
                            nc.vector.tensor_tensor(
                                s[:, 0:nb, :],
                                meta_sb[:, b0:b0 + nb].unsqueeze(2)
                                .broadcast_to([P, nb, P]),
                                iota[:, :].unsqueeze(1).broadcast_to(
                                    [P, nb, P]),
                                mybir.AluOpType.is_equal,
                            )
                            for j in range(nb):
                                st = (bi == 0)
                                sp = (bi == nblk_w - 1)
                                nc.tensor.matmul(pn_t[:, :], s[:, j, :],
                                                 g[:, j, 0:TD],
                                                 start=st, stop=sp,
                                                 skip_group_check=True)
                                nc.tensor.matmul(pn_u[:, :], s[:, j, :],
                                                 g[:, j, TD:TD + HEADS],
                                                 start=st, stop=sp,
                                                 skip_group_check=True)
                                bi += 1
                    if h == 0:
                        # stash pass-0 partials in SBUF accumulator (on the
                        # mostly-idle Scalar engine; ACT can read PSUM)
                        nc.scalar.activation(
                            acc_t[:, w, :], pn_t[:, :],
                            mybir.ActivationFunctionType.Identity)
                        nc.scalar.activation(
                            acc_u[:, w, :], pn_u[:, :],
                            mybir.ActivationFunctionType.Identity)
                    else:
                        # ---- evict window ----
                        den = opool.tile([P, HEADS], F32, tag="den")
                        nc.vector.scalar_tensor_tensor(
                            out=den[:, :], in0=pn_u[:, :], scalar=1e-30,
                            in1=acc_u[:, w, :],
                            op0=mybir.AluOpType.add, op1=mybir.AluOpType.add)
                        rden = opool.tile([P, HEADS], F32, tag="rden")
                        nc.vector.reciprocal(rden[:, :], den[:, :])
                        num = opool.tile([P, TD], F32, tag="num")
                        nc.vector.tensor_tensor(
                            num[:, :], pn_t[:, :], acc_t[:, w, :],
                            mybir.AluOpType.add)
                        hout = opool.tile([P, TD], F32, tag="hout")
                        nc.vector.tensor_tensor(
                            hout[:, :].rearrange("p (h o) -> p h o", h=HEADS),
                            num[:, :].rearrange("p (h o) -> p h o", h=HEADS),
                            rden[:, :].unsqueeze(2).broadcast_to(
                                [P, HEADS, OUT]),
                            mybir.AluOpType.mult,
                        )
                        # elu(z) = max(z,0) + exp(min(z,0)) - 1
                        xm = opool.tile([P, TD], F32, tag="xm")
                        nc.scalar.activation(xm[:, :], hout[:, :],
                                             mybir.ActivationFunctionType.Relu,
                                             scale=-1.0)
                        ex = opool.tile([P, TD], F32, tag="ex")
                        nc.scalar.activation(ex[:, :], xm[:, :],
                                             mybir.ActivationFunctionType.Exp,
                                             scale=-1.0)
                        fin = opool.tile([P, TD], F32, tag="fin")
                        nc.vector.scalar_tensor_tensor(
                            out=fin[:, :], in0=hout[:, :], scalar=0.0,
                            in1=ex[:, :],
                            op0=mybir.AluOpType.max, op1=mybir.AluOpType.add)
                        # reuse `num` (dead after hout) for the final output
                        nc.scalar.activation(
                            num[:, :], fin[:, :],
                            mybir.ActivationFunctionType.Identity,
                            bias=neg1[:, :])
                        nc.sync.dma_start(out_d[:, w, :], num[:, :])

    nc.compile()
    return nc


def host_prep(cfg: Config, x, Ws, As):
    import ml_dtypes
    x = np.asarray(x, np.float32)
    Ws = np.asarray(Ws, np.float32)
    As = np.asarray(As, np.float32)
    n = x.shape[0]
    xpad = np.zeros((N_PAD, IN_FEAT), np.float32)
    xpad[:n] = x
    # xt[p, t, k, j] = x[t*128 + j, k*128 + p]
    xt = np.ascontiguousarray(
        xpad.reshape(N_TILES, P, 2, 128).transpose(3, 0, 2, 1)
    ).astype(ml_dtypes.bfloat16)
    wcat = Ws.transpose(2, 0, 1).reshape(IN_FEAT, TD).astype(
        ml_dtypes.bfloat16)
    a_r = As[:, OUT:, 0]
    war = np.einsum("hof,ho->fh", Ws, a_r).astype(ml_dtypes.bfloat16)
    iota = np.tile(np.arange(P, dtype=np.float32), (P, 1)).astype(
        ml_dtypes.bfloat16)
    meta = cfg.meta_packed.astype(ml_dtypes.bfloat16)
    in_maps = []
    for c in range(N_CORES):
        m = {
            "xt": xt, "wcat": wcat, "war": war,
            "iota": np.ascontiguousarray(iota),
            "meta": np.ascontiguousarray(meta[c]),
        }
        if GATHER_MODE == "indirect":
            m["idx32"] = np.ascontiguousarray(cfg.idx32_packed[c])
        else:
            m["idx"] = np.ascontiguousarray(cfg.idx_packed[c])
        in_maps.append(m)
    return in_maps


from concourse.bass_utils import run_bass_kernel_spmd

LAST_EXEC_TIME_NS = None


def kernel(x, src, dst, Ws, As):
    """Full-input entry point: shards internally across 8 NeuronCores."""
    global LAST_EXEC_TIME_NS
    x = np.asarray(x, np.float32)
    src = np.asarray(src)
    dst = np.asarray(dst)
    Ws = np.asarray(Ws, np.float32)
    As = np.asarray(As, np.float32)
    n = x.shape[0]

    cfg = Config(n, src, dst)
    nc = build_program(cfg)
    in_maps = host_prep(cfg, x, Ws, As)
    import os as _os
    _trace = _os.environ.get("KERNEL_TRACE", "0") == "1"
    _tdir = _os.environ.get("KERNEL_TRACE_DIR") or None
    if _tdir:
        _os.makedirs(_tdir, exist_ok=True)
    res = run_bass_kernel_spmd(nc, in_maps, core_ids=list(range(N_CORES)),
                               trace=_trace, tmpdir=_tdir)
    LAST_EXEC_TIME_NS = res.exec_time_ns
    # out[p, w, :] holds node (w*128 + p) of each core's range
    outs = []
    for c in range(N_CORES):
        o = res.results[c]["out"]          # [128, 49, 512]
        outs.append(np.ascontiguousarray(o.transpose(1, 0, 2)).reshape(
            NPC, TD))
    out = np.concatenate(outs, axis=0)[:n]
    return np.ascontiguousarray(out, dtype=np.float32)
